# revision 21
# baseline (speedup 1.0000x reference)
"""AttentionLM Trainium2 kernel: 8-way sharded (head-parallel attention +
vocab-sharded output projection with an on-chip AllGather in between).

Contract: kernel(**inputs) takes the FULL inputs from reference.setup_inputs()
and returns the FULL [B, S, VOCAB] fp32 logits.
"""

import os
import sys

for _p in ("/opt/trn_rl_repo",):
    if _p not in sys.path:
        sys.path.insert(0, _p)

import numpy as np

import concourse.bass as bass
import concourse.mybir as mybir
import concourse.tile as tile
from concourse import bacc
from concourse.bass import IndirectOffsetOnAxis
from concourse.bass_utils import run_bass_kernel_spmd

# Problem shape (hardcoded per contract)
B, S = 2, 2048
VOCAB = 32000
E = 1024
H = 16
D = 64

N_CORES = 8
HPC = H // N_CORES          # heads per core = 2
VS = VOCAB // N_CORES       # vocab shard = 4000
BS = B * S                  # 4096 flattened tokens
P = 128
ST = BS // P                # 32 token tiles
ET = E // P                 # 8 embed tiles
SBLK = 512                  # token block for matmul moving dim
NSB = BS // SBLK            # 8 token blocks
SPB = S // SBLK             # 4 token blocks per batch
TTB = S // P                # 16 key tiles per batch
VBW = 512                   # vocab block width
NVB = (VS + VBW - 1) // VBW  # 8 vocab blocks (last = 416)

f32 = mybir.dt.float32
i32 = mybir.dt.int32
AF = mybir.ActivationFunctionType
ALU = mybir.AluOpType

# matmul input dtype: float32r (tf32) streams at full PE rate for
# free-dim >= 256; float32 runs at 1/4 rate.
MM_DT = f32 if os.environ.get("KMM_DT", "f32r") == "f32" else mybir.dt.float32r
KPH = os.environ.get("KPH", "full")  # A | AB | ABG | full (debug bisect)


def build_nc():
    nc = bacc.Bacc("TRN2", target_bir_lowering=False, debug=False,
                   num_devices=N_CORES)

    tok = nc.dram_tensor("tok", [P, ST], i32, kind="ExternalInput")
    emb = nc.dram_tensor("emb", [VOCAB, E], f32, kind="ExternalInput")
    pos = nc.dram_tensor("pos", [S, E], f32, kind="ExternalInput")
    wq = nc.dram_tensor("wq", [E, P], f32, kind="ExternalInput")
    wk = nc.dram_tensor("wk", [E, P], f32, kind="ExternalInput")
    wv = nc.dram_tensor("wv", [E, P], f32, kind="ExternalInput")
    linw = nc.dram_tensor("linw", [E, VS], f32, kind="ExternalInput")
    bias = nc.dram_tensor("bias", [P, VS], f32, kind="ExternalInput")
    ident = nc.dram_tensor("ident", [P, P], f32, kind="ExternalInput")
    out = nc.dram_tensor("out", [BS, VS], f32, kind="ExternalOutput")

    with tile.TileContext(nc) as tc:
        with tc.tile_pool(name="dram", bufs=1, space="DRAM") as dram:
            zT_loc = dram.tile([P, BS], MM_DT)
            zT_full = dram.tile([P * N_CORES, BS], MM_DT, addr_space="Shared")
            sums_dram = dram.tile([16, SBLK], f32)

            pp_ctx = tc.tile_pool(name="persist", bufs=1)
            pp = pp_ctx.__enter__()
            # persistent SBUF tensors for phases A+B
            tok_sb = pp.tile([P, ST], i32)
            ident_sb = pp.tile([P, P], f32)
            wq_sb = pp.tile([P, ET, P], MM_DT)
            wk_sb = pp.tile([P, ET, P], MM_DT)
            wv_sb = pp.tile([P, ET, P], MM_DT)
            qT_sb = pp.tile([P, BS], MM_DT)    # [2 heads * 64 d, token]
            kT_sb = pp.tile([P, BS], MM_DT)
            vT_sb = pp.tile([P, BS], f32)
            v_all = pp.tile([P, ST, 130], MM_DT)  # [t in tile, t-tile, d-aug]
            zT_pair = pp.tile([P, BS], f32)
            zT_norm = pp.tile([P, BS], MM_DT)
            sums_bc = pp.tile([P, BS], f32)

            nc.sync.dma_start(tok_sb[:], tok[:])
            nc.sync.dma_start(ident_sb[:], ident[:])
            with tc.tile_pool(name="wstage", bufs=1) as wst:
                wq_st = wst.tile([P, ET, P], f32, name="wq_st")
                wk_st = wst.tile([P, ET, P], f32, name="wk_st")
                wv_st = wst.tile([P, ET, P], f32, name="wv_st")
                for w_dram, w_st in ((wq, wq_st), (wk, wk_st), (wv, wv_st)):
                    nc.sync.dma_start(
                        w_st[:],
                        w_dram[:].rearrange("(et p) d -> p et d", p=P))
                # round to fp32r (tf32) for the PE
                nc.vector.tensor_copy(wq_sb[:], wq_st[:])
                nc.vector.tensor_copy(wk_sb[:], wk_st[:])
                nc.vector.tensor_copy(wv_sb[:], wv_st[:])
            # augmented ones columns for attention denominator rows
            with tc.tile_pool(name="onesp", bufs=1) as onesp:
                ones_st = onesp.tile([P, ST, 1], f32)
                nc.vector.memset(ones_st[:], 1.0)
                nc.vector.tensor_copy(v_all[:, :, 64:65], ones_st[:])
                nc.vector.tensor_copy(v_all[:, :, 129:130], ones_st[:])

            # ---------------- Phase A: embed + gelu + QKV ----------------
            with tc.tile_pool(name="raw", bufs=3) as rawp, \
                 tc.tile_pool(name="posp", bufs=3) as posp, \
                 tc.tile_pool(name="hpp", bufs=5) as hpp, \
                 tc.tile_pool(name="htc", bufs=9) as htp, \
                 tc.tile_pool(name="psA", bufs=2, space="PSUM") as psA, \
                 tc.tile_pool(name="psQ", bufs=3, space="PSUM") as psQ, \
                 tc.tile_pool(name="psV", bufs=2, space="PSUM") as psV:
                drain_flip = 0
                for sb in range(NSB):
                    hps = []
                    for j in range(4):
                        idx = sb * 4 + j
                        raw = rawp.tile([P, E], f32, tag="raw")
                        nc.gpsimd.indirect_dma_start(
                            out=raw[:],
                            out_offset=None,
                            in_=emb[:],
                            in_offset=IndirectOffsetOnAxis(
                                ap=tok_sb[:, idx:idx + 1], axis=0),
                        )
                        pos_t = posp.tile([P, E], f32, tag="pos")
                        pr = (idx % (S // P)) * P
                        nc.sync.dma_start(pos_t[:], pos[pr:pr + P, :])
                        hp = hpp.tile([P, E], f32, tag="hp")
                        nc.vector.tensor_tensor(hp[:], raw[:], pos_t[:],
                                                op=ALU.add)
                        nc.scalar.activation(hp[:], hp[:], AF.Gelu)
                        hps.append(hp)
                    # transpose h into [e, token] layout (+ fp32r round)
                    htc = []
                    for et in range(ET):
                        ps = psA.tile([P, SBLK], f32, tag="pst")
                        for j in range(4):
                            nc.tensor.transpose(
                                ps[:, j * P:(j + 1) * P],
                                hps[j][:, et * P:(et + 1) * P],
                                ident_sb[:],
                            )
                        hc = htp.tile([P, SBLK], MM_DT, tag="htc")
                        if drain_flip % 2 == 0:
                            nc.vector.tensor_copy(hc[:], ps[:])
                        else:
                            nc.scalar.copy(hc[:], ps[:])
                        drain_flip += 1
                        htc.append(hc)
                    # q, k, v projections: [128 d2, 512 tokens]
                    for w_sb, dstT in ((wq_sb, qT_sb), (wk_sb, kT_sb),
                                       (wv_sb, vT_sb)):
                        psq = psQ.tile([P, SBLK], f32, tag="psq")
                        for et in range(ET):
                            nc.tensor.matmul(
                                psq[:],
                                lhsT=w_sb[:, et, :],
                                rhs=htc[et][:],
                                start=(et == 0), stop=(et == ET - 1),
                            )
                        col = sb * SBLK
                        if drain_flip % 2 == 0:
                            nc.vector.tensor_copy(dstT[:, col:col + SBLK],
                                                  psq[:])
                        else:
                            nc.scalar.copy(dstT[:, col:col + SBLK], psq[:])
                        drain_flip += 1

                # v into [token, d-aug] layout via PE transpose
                for bt in range(ST):
                    psv = psV.tile([P, P], f32, tag="psv")
                    nc.tensor.transpose(
                        psv[:],
                        vT_sb[:, bt * P:(bt + 1) * P],
                        ident_sb[:],
                    )
                    nc.vector.tensor_copy(v_all[:, bt, 0:64], psv[:, 0:64])
                    nc.vector.tensor_copy(v_all[:, bt, 65:129],
                                          psv[:, 64:128])

            # ---------------- Phase B: attention ----------------
            if KPH != "A":
                with tc.tile_pool(name="expp", bufs=20) as expp, \
                     tc.tile_pool(name="sump", bufs=3) as sump, \
                     tc.tile_pool(name="psS", bufs=4, space="PSUM") as psS, \
                     tc.tile_pool(name="psZ", bufs=2, space="PSUM") as psZ:
                    for b in range(B):
                        for h in range(HPC):
                            for sblk in range(SPB):
                                qcol = b * S + sblk * SBLK
                                psz = psZ.tile([P, SBLK], f32, tag="psz")
                                for tt in range(TTB):
                                    tcol = b * S + tt * P
                                    pss = psS.tile([P, SBLK], f32, tag="pss")
                                    nc.tensor.matmul(
                                        pss[:],
                                        lhsT=kT_sb[64 * h:64 * h + 64,
                                                   tcol:tcol + P],
                                        rhs=qT_sb[64 * h:64 * h + 64,
                                                  qcol:qcol + SBLK],
                                        start=True, stop=True,
                                    )
                                    ex = expp.tile([P, SBLK], MM_DT, tag="ex")
                                    nc.scalar.activation(ex[:], pss[:],
                                                         AF.Exp,
                                                         scale=1.0 / D)
                                    nc.tensor.matmul(
                                        psz[:65, :],
                                        lhsT=v_all[:, b * TTB + tt,
                                                   65 * h:65 * h + 65],
                                        rhs=ex[:],
                                        start=(tt == 0),
                                        stop=(tt == TTB - 1),
                                    )
                                pidx = h * 8 + b * 4 + sblk
                                nc.vector.tensor_copy(
                                    zT_pair[64 * h:64 * h + 64,
                                            qcol:qcol + SBLK],
                                    psz[0:64, :])
                                srow = sump.tile([1, SBLK], f32, tag="srow")
                                nc.vector.tensor_copy(srow[:], psz[64:65, :])
                                nc.sync.dma_start(
                                    sums_dram[pidx:pidx + 1, :], srow[:])

                    # normalize: broadcast sums across partitions via DRAM
                    r0 = sums_dram[0:8, :].rearrange("a b -> (a b)") \
                        .rearrange("(o a) -> o a", o=1)
                    r1 = sums_dram[8:16, :].rearrange("a b -> (a b)") \
                        .rearrange("(o a) -> o a", o=1)
                    nc.sync.dma_start(sums_bc[0:64, :],
                                      r0.to_broadcast((64, BS)))
                    nc.sync.dma_start(sums_bc[64:128, :],
                                      r1.to_broadcast((64, BS)))
                    nc.vector.reciprocal(sums_bc[:], sums_bc[:])
                    nc.vector.tensor_tensor(zT_norm[:], zT_pair[:],
                                            sums_bc[:], op=ALU.mult)

            if KPH in ("ABG", "full"):
                # gather z across cores: rows 128c:+128 = heads (2c, 2c+1)
                nc.sync.dma_start(zT_loc[:], zT_norm[:])
                nc.gpsimd.collective_compute(
                    "AllGather",
                    ALU.bypass,
                    replica_groups=[list(range(N_CORES))],
                    ins=[zT_loc.opt()],
                    outs=[zT_full.opt()],
                )

            # release phase A/B SBUF before phase C
            pp_ctx.__exit__(None, None, None)

            # ---------------- Phase C: output projection ----------------
            if KPH == "full":
                with tc.tile_pool(name="lwp", bufs=1) as lwp, \
                     tc.tile_pool(name="lwstage", bufs=2) as lws, \
                     tc.tile_pool(name="ztp", bufs=3) as ztp, \
                     tc.tile_pool(name="biasp", bufs=1) as biasp, \
                     tc.tile_pool(name="outp", bufs=6) as outp, \
                     tc.tile_pool(name="psO", bufs=8, space="PSUM") as psO:
                    lw_all = lwp.tile([P, ET, VS], MM_DT)
                    for vb in range(NVB):
                        off = vb * VBW
                        wid = min(VBW, VS - off)
                        stg = lws.tile([P, ET, VBW], f32, tag="lwstg")
                        nc.sync.dma_start(
                            stg[:, :, :wid],
                            linw[:, off:off + wid]
                            .rearrange("(et p) d -> p et d", p=P))
                        for et in range(ET):
                            nc.vector.tensor_copy(
                                lw_all[:, et, off:off + wid],
                                stg[:, et, :wid])
                    bias_sb = biasp.tile([P, VS], f32)
                    nc.sync.dma_start(bias_sb[:], bias[:])

                    for st in range(ST):
                        zt_st = ztp.tile([P, ET, P], MM_DT, tag="zt")
                        nc.sync.dma_start(
                            zt_st[:],
                            zT_full[:, st * P:(st + 1) * P]
                            .rearrange("(et p) d -> p et d", p=P))
                        psos = [psO.tile([P, VBW], f32, tag="pso",
                                         name=f"pso_{st}_{vb}")
                                for vb in range(NVB)]
                        for et in range(ET):
                            for vb in range(NVB):
                                off = vb * VBW
                                wid = min(VBW, VS - off)
                                nc.tensor.matmul(
                                    psos[vb][:, :wid],
                                    lhsT=zt_st[:, et, :],
                                    rhs=lw_all[:, et, off:off + wid],
                                    start=(et == 0), stop=(et == ET - 1),
                                )
                        for vb in range(NVB):
                            off = vb * VBW
                            wid = min(VBW, VS - off)
                            tmp = outp.tile([P, VBW], f32, tag="tmp")
                            nc.vector.tensor_tensor(
                                tmp[:, :wid], psos[vb][:, :wid],
                                bias_sb[:, off:off + wid], op=ALU.add)
                            nc.scalar.activation(tmp[:, :wid], tmp[:, :wid],
                                                 AF.Relu)
                            nc.sync.dma_start(
                                out[st * P:(st + 1) * P, off:off + wid],
                                tmp[:, :wid])
    nc.compile()
    return nc


_NC_CACHE = None


def get_nc():
    global _NC_CACHE
    if _NC_CACHE is None:
        _NC_CACHE = build_nc()
    return _NC_CACHE


def make_in_maps(x, embed_table, pos_table, wq, wk, wv, lin_w, lin_b):
    x = np.asarray(x).reshape(-1).astype(np.int32)
    embed_table = np.ascontiguousarray(np.asarray(embed_table,
                                                  dtype=np.float32))
    pos_table = np.ascontiguousarray(
        np.asarray(pos_table, dtype=np.float32)[:S])
    wq = np.asarray(wq, dtype=np.float32)
    wk = np.asarray(wk, dtype=np.float32)
    wv = np.asarray(wv, dtype=np.float32)
    lin_w = np.asarray(lin_w, dtype=np.float32)
    lin_b = np.asarray(lin_b, dtype=np.float32)

    tok = np.ascontiguousarray(x.reshape(ST, P).T)  # tok[p, i] = x[i*128+p]
    ident = np.eye(P, dtype=np.float32)

    in_maps = []
    for c in range(N_CORES):
        h0 = HPC * c
        wq_p = np.ascontiguousarray(
            np.concatenate([wq[h0 + j] for j in range(HPC)], axis=1))
        wk_p = np.ascontiguousarray(
            np.concatenate([wk[h0 + j] for j in range(HPC)], axis=1))
        wv_p = np.ascontiguousarray(
            np.concatenate([wv[h0 + j] for j in range(HPC)], axis=1))
        lw = np.ascontiguousarray(lin_w[:, VS * c:VS * (c + 1)])
        bb = np.ascontiguousarray(
            np.broadcast_to(lin_b[VS * c:VS * (c + 1)], (P, VS)))
        in_maps.append({
            "tok": tok, "emb": embed_table, "pos": pos_table,
            "wq": wq_p, "wk": wk_p, "wv": wv_p,
            "linw": lw, "bias": bb, "ident": ident,
        })
    return in_maps


def run(in_maps, trace=False):
    nc = get_nc()
    return run_bass_kernel_spmd(nc, in_maps, core_ids=list(range(N_CORES)),
                                trace=trace)


def kernel(x, embed_table, pos_table, wq, wk, wv, lin_w, lin_b):
    in_maps = make_in_maps(x, embed_table, pos_table, wq, wk, wv, lin_w, lin_b)
    res = run(in_maps)
    logits = np.empty((B, S, VOCAB), dtype=np.float32)
    for c in range(N_CORES):
        logits[:, :, VS * c:VS * (c + 1)] = \
            res.results[c]["out"].reshape(B, S, VS)
    return logits
